# revision 5
# baseline (speedup 1.0000x reference)
"""FASTMultiHeadAttention (polynomial softmax + RPE bias, causal) on 8 trn2 cores.

Math per (b,h):   w[i,j] = q_i.k_j + q_i.rpe[n-1+i-j] + 1
                  score  = 0.5*(w^2 + 1)     (0.5 cancels in num/den)
                  o_i    = (sum_{j<=i} w^2 v_j + cumsum(v)_i) / denom-col

All operands run in fp16 (same PE speed as bf16, 4x the mantissa -> rel err
~1.7e-3 vs the 2e-2 gate). Per core (4 (b,h) units), per 128-row tile I:

  - band matmul  m2r[p,u] = q . rpeR[u0+u]   (PE rows 0-63, tile_position (0,0))
  - s matmul     s[p,m]   = q . k            (PE rows 64-127, tile_position (64,0))
  - ACT evac     band psum -> m2rS sbuf (fp16)
  - shear DMA    bias[p,m] = m2rS[p, 127+c-p+m]  (SBUF->SBUF, one per row tile)
  - DVE stt      w = (s_psum + 1) + bias -> fp16
  - PE transpose w as 64-row HALF-blocks on the two tile positions
    (pairs execute concurrently -> ~2x transpose throughput), halves land
    in two separate full-bank psum tiles
  - ACT Square (a-halves) / DVE copy+mult (b-halves)  scoreT = wT^2 -> sbuf
  - GP mask      causal mask on transposed diag block (affine_select)
  - PE o-matmul  o[i,u] += scoreT_J.T @ va_J   (row-major out, M=128 N=65;
                 kills the transpose-back + per-tile divide of the baseline)
  - DVE          oaug = o_psum + cumsum(va) (batched over 4 row tiles),
                 recip of denom cols, broadcast multiply -> o_fin

The A (score-row), B (transpose), C (output) stages are software-pipelined
GLOBALLY over all 32 row tiles (4 bh x 8), so one bh unit's tail overlaps the
next unit's head and input DMAs prefetch ahead of the store in queue order.
Tiles are processed big-first ([4,5,6,7,0,1,2,3]) so the pipeline fills with
PE work while the first evac->shear->stt chain completes.
"""

import sys

if "/opt/trn_rl_repo" not in sys.path:
    sys.path.insert(0, "/opt/trn_rl_repo")

import ml_dtypes
import numpy as np

import bass_rust
import concourse.bacc as bacc
import concourse.mybir as mybir
import concourse.tile as tile
from concourse.bass_utils import run_bass_kernel_spmd

F32 = mybir.dt.float32
F32R = mybir.dt.float32r
BF16 = mybir.dt.bfloat16
F16 = mybir.dt.float16

B, H, N, D = 2, 16, 1024, 64
NBH = B * H
N_CORES = 8
BH_PER_CORE = NBH // N_CORES  # 4
NT = N // 128  # 8 row tiles
RPE_W = 1408  # rpeR width (1151 band + f32r pad)
M2RS_W = 1152


def _chunks(total, pad_min=256):
    """Split [0,total) at 512; return (start, width, padded_width)."""
    out = []
    c = 0
    while c < total:
        wd = min(512, total - c)
        out.append((c, wd, max(wd, pad_min)))
        c += 512
    return out


S_CHUNKS = {I: _chunks(128 * (I + 1)) for I in range(NT)}
B_CHUNKS = {I: _chunks(255 + 128 * I) for I in range(NT)}
U0 = {I: 896 - 128 * I for I in range(NT)}


def _ap(t_ap, pairs, offset=0):
    cp = t_ap.copy()
    cp.ap = bass_rust.VecI64Pair(pairs)
    cp.offset = offset
    return cp


def _shear_ap(t_ap, row_elems, offset, width):
    """AP reading t[p, offset - p + m] for m in [0, width)."""
    return _ap(t_ap, [[row_elems - 1, 128], [1, width]], offset)


def build_program():
    nc = bacc.Bacc(
        "TRN2", target_bir_lowering=False, debug=False, num_devices=N_CORES
    )

    qT_d = nc.dram_tensor("qT", [BH_PER_CORE, 64, N], F16, kind="ExternalInput").ap()
    kT_d = nc.dram_tensor("kT", [BH_PER_CORE, 64, N], F16, kind="ExternalInput").ap()
    va_d = nc.dram_tensor("va", [BH_PER_CORE, N, 65], F16, kind="ExternalInput").ap()
    pt_d = nc.dram_tensor("ptc", [BH_PER_CORE, N, 65], F32, kind="ExternalInput").ap()
    rpe_d = nc.dram_tensor("rpeR", [64, RPE_W], F16, kind="ExternalInput").ap()
    idn_d = nc.dram_tensor("idn", [128, 128], F16, kind="ExternalInput").ap()
    o_d = nc.dram_tensor("o", [BH_PER_CORE, N, 64], F32, kind="ExternalOutput").ap()

    with tile.TileContext(nc) as tc:
        with (
            tc.tile_pool(name="const", bufs=1) as cpool,
            tc.tile_pool(name="io", bufs=3) as io,
            tc.tile_pool(name="m2rs", bufs=2) as m2rp,
            tc.tile_pool(name="bias", bufs=3) as bp,
            tc.tile_pool(name="wrow", bufs=3) as wp,
            tc.tile_pool(name="sct", bufs=4) as scp,
            tc.tile_pool(name="oau", bufs=2) as op,
            tc.tile_pool(name="fin", bufs=2) as fp,
            tc.tile_pool(name="psb", bufs=3, space="PSUM") as ps_b,
            tc.tile_pool(name="pss", bufs=2, space="PSUM") as ps_s,
            tc.tile_pool(name="pswta", bufs=1, space="PSUM") as ps_wta,
            tc.tile_pool(name="pswtb", bufs=1, space="PSUM") as ps_wtb,
            tc.tile_pool(name="pso", bufs=1, space="PSUM") as ps_o,
        ):
            rpeR = cpool.tile([64, RPE_W], F16)
            nc.sync.dma_start(rpeR[:], rpe_d[:])
            idn = cpool.tile([128, 128], F16)
            nc.sync.dma_start(idn[:], idn_d[:])

            bhs = {}
            st_w = {}
            st_sc = {}
            st_po = {}

            def load_bh(m):
                # q on both PE row-halves; k on rows 64-127 only
                qT = io.tile([128, N], F16, tag="qT", name="qT")
                nc.sync.dma_start(qT[0:64, :], qT_d[m])
                nc.sync.dma_start(qT[64:128, :], qT_d[m])
                kT = io.tile([128, N], F16, tag="kT", name="kT")
                nc.sync.dma_start(kT[64:128, :], kT_d[m])
                va = io.tile([128, NT * 65], F16, tag="va", name="va")
                nc.sync.dma_start(
                    va[:].rearrange("p (a d) -> p a d", a=NT),
                    va_d[m].rearrange("(a b) d -> b a d", a=NT),
                )
                ptc = io.tile([128, NT * 65], F32, tag="ptc", name="ptc")
                nc.sync.dma_start(
                    ptc[:].rearrange("p (a d) -> p a d", a=NT),
                    pt_d[m].rearrange("(a b) d -> b a d", a=NT),
                )
                o_fin = fp.tile([128, NT * 64], F32, tag="ofin", name="o_fin")
                bhs[m] = (qT, kT, va, ptc, o_fin)

            def stage_a(m, I):
                qT, kT, va, ptc, o_fin = bhs[m]
                u0 = U0[I]
                W = 128 * (I + 1)
                m2rS = m2rp.tile([128, M2RS_W], F16, tag="m2rs", name="m2rS")
                wrow = wp.tile([128, N], F16, tag="wrow", name="wrow")
                st_w[(m, I)] = wrow

                bch = B_CHUNKS[I]
                sch = S_CHUNKS[I]
                pbs = []
                pses = []
                for ci in range(max(len(bch), len(sch))):
                    if ci < len(bch):
                        c, wd, wdp = bch[ci]
                        pb = ps_b.tile([128, 512], F32, tag="pb", name="pb")
                        nc.tensor.matmul(
                            pb[:, :wdp],
                            qT[0:64, 128 * I : 128 * (I + 1)],
                            rpeR[:, u0 + c : u0 + c + wdp],
                            start=True,
                            stop=True,
                            tile_position=(0, 0),
                        )
                        pbs.append((c, wd, pb))
                    if ci < len(sch):
                        c, wd, wdp = sch[ci]
                        ss = ps_s.tile([128, 512], F32, tag="ss", name="ss")
                        nc.tensor.matmul(
                            ss[:, :wdp],
                            qT[64:128, 128 * I : 128 * (I + 1)],
                            kT[64:128, c : c + wdp],
                            start=True,
                            stop=True,
                            tile_position=(64, 0),
                        )
                        pses.append((c, wd, ss))
                    if ci < len(pbs):
                        c, wd, pb = pbs[ci]
                        nc.scalar.copy(m2rS[:, c : c + wd], pb[:, :wd])

                bias = bp.tile([128, 1024], F16, tag="bias", name="bias")
                nc.sync.dma_start(
                    bias[:, :W],
                    _shear_ap(m2rS[:], M2RS_W, 127, W),
                )
                for c, wd, ss in pses:
                    nc.vector.scalar_tensor_tensor(
                        wrow[:, c : c + wd],
                        ss[:, :wd],
                        1.0,
                        bias[:, c : c + wd],
                        mybir.AluOpType.add,
                        mybir.AluOpType.add,
                    )

            def stage_b(m, I):
                wrow = st_w[(m, I)]
                nb = I + 1
                scT = scp.tile([128, N], F16, tag="scT", name="scT")
                st_sc[(m, I)] = scT
                # 64-row half-transposes on the two PE tile positions
                # (pairs can execute concurrently); halves land in two
                # full-bank psum tiles.
                pwa = ps_wta.tile([128, 1024], F16, tag="pwa", name="pwa")
                pwb = ps_wtb.tile([128, 1024], F16, tag="pwb", name="pwb")
                for J in range(nb):
                    nc.tensor.transpose(
                        pwa[:, 64 * J : 64 * (J + 1)],
                        wrow[0:64, 128 * J : 128 * (J + 1)],
                        idn[0:64, 0:64],
                        tile_position=(0, 0),
                    )
                    nc.tensor.transpose(
                        pwb[:, 64 * J : 64 * (J + 1)],
                        wrow[64:128, 128 * J : 128 * (J + 1)],
                        idn[64:128, 64:128],
                        tile_position=(64, 0),
                    )
                # scoreT = Square(wT): ACT takes the a-halves, DVE the b-halves
                nc.scalar.activation(
                    _ap(scT[:], [[NT * 128, 128], [128, nb], [1, 64]], offset=0),
                    _ap(pwa[:], [[1024, 128], [64, nb], [1, 64]], offset=0),
                    mybir.ActivationFunctionType.Square,
                )
                nc.vector.tensor_copy(
                    _ap(scT[:], [[NT * 128, 128], [128, nb], [1, 64]], offset=64),
                    _ap(pwb[:], [[1024, 128], [64, nb], [1, 64]], offset=0),
                )
                nc.vector.tensor_tensor(
                    _ap(scT[:], [[NT * 128, 128], [128, nb], [1, 64]], offset=64),
                    _ap(scT[:], [[NT * 128, 128], [128, nb], [1, 64]], offset=64),
                    _ap(scT[:], [[NT * 128, 128], [128, nb], [1, 64]], offset=64),
                    mybir.AluOpType.mult,
                )
                # causal mask on transposed diag block: keep ii >= jj
                nc.gpsimd.affine_select(
                    scT[:, 128 * I : 128 * (I + 1)],
                    scT[:, 128 * I : 128 * (I + 1)],
                    pattern=[[1, 128]],
                    compare_op=mybir.AluOpType.is_ge,
                    fill=0.0,
                    base=0,
                    channel_multiplier=-1,
                )

            def stage_c(m, I):
                qT, kT, va, ptc, o_fin = bhs[m]
                scT = st_sc[(m, I)]
                g = I // 4
                if I % 4 == 0:
                    st_po[(m, g)] = ps_o.tile([128, 260], F32, tag="po", name="po")
                po = st_po[(m, g)]
                s0 = 65 * (I % 4)
                for J in range(I + 1):
                    nc.tensor.matmul(
                        po[:, s0 : s0 + 65],
                        scT[:, 128 * J : 128 * (J + 1)],
                        va[:, 65 * J : 65 * (J + 1)],
                        start=(J == 0),
                        stop=(J == I),
                    )
                if I % 4 == 3:
                    p4 = g
                    oaug = op.tile([128, 260], F32, tag="oaug", name="oaug")
                    nc.vector.scalar_tensor_tensor(
                        oaug[:],
                        po[:],
                        1.0,
                        ptc[:, 260 * p4 : 260 * (p4 + 1)],
                        mybir.AluOpType.mult,
                        mybir.AluOpType.add,
                    )
                    rc = op.tile([128, 4], F32, tag="rc", name="rc")
                    nc.vector.reciprocal(
                        rc[:], _ap(oaug[:], [[260, 128], [65, 4]], offset=64)
                    )
                    nc.vector.tensor_tensor(
                        _ap(o_fin[:], [[NT * 64, 128], [64, 4], [1, 64]],
                            offset=256 * p4),
                        _ap(oaug[:], [[260, 128], [65, 4], [1, 64]]),
                        _ap(rc[:], [[4, 128], [1, 4], [0, 64]]),
                        mybir.AluOpType.mult,
                    )
                    if I == ORDER[-1]:
                        nc.sync.dma_start(
                            o_d[m].rearrange("(a b) d -> b a d", a=NT),
                            o_fin[:].rearrange("p (a d) -> p a d", a=NT),
                        )

            # big tiles first within each bh: fills the pipeline with PE
            # work while the evac->shear->stt chain of the first tile runs;
            # po/oaug groups {4..7}, {0..3} stay contiguous.
            ORDER = [4, 5, 6, 7, 0, 1, 2, 3]
            TOT = BH_PER_CORE * NT
            for gt in range(TOT + 3):
                if gt < TOT:
                    m, pos = divmod(gt, NT)
                    if pos == 0:
                        load_bh(m)
                    stage_a(m, ORDER[pos])
                if 1 <= gt <= TOT:
                    m, pos = divmod(gt - 1, NT)
                    stage_b(m, ORDER[pos])
                if gt >= 3:
                    m, pos = divmod(gt - 3, NT)
                    stage_c(m, ORDER[pos])

    nc.compile()
    return nc


_NC_CACHE = {}


def get_program():
    if "nc" not in _NC_CACHE:
        _NC_CACHE["nc"] = build_program()
    return _NC_CACHE["nc"]


def prepare_inputs(q, k, v, rpe_matrix):
    q = np.asarray(q, dtype=np.float32).reshape(NBH, N, D)
    k = np.asarray(k, dtype=np.float32).reshape(NBH, N, D)
    v = np.asarray(v, dtype=np.float32).reshape(NBH, N, D)
    rpe = np.asarray(rpe_matrix, dtype=np.float32)

    qT = np.ascontiguousarray(q.transpose(0, 2, 1)).astype(np.float16)
    kT = np.ascontiguousarray(k.transpose(0, 2, 1)).astype(np.float16)
    va32 = np.concatenate([v, np.ones((NBH, N, 1), np.float32)], axis=2)
    va = va32.astype(np.float16)  # [32, 1024, 65] fp16
    # cumsum of fp16-rounded va, in f64 for exactness, as f32
    ptc = np.cumsum(va.astype(np.float64), axis=1).astype(np.float32)

    rpeR = np.zeros((64, RPE_W), np.float16)
    rpeR[:, :1151] = rpe[2046:895:-1].T.astype(np.float16)
    idn = np.eye(128, dtype=np.float16)


    in_maps = []
    for c in range(N_CORES):
        sl = slice(c * BH_PER_CORE, (c + 1) * BH_PER_CORE)
        in_maps.append(
            {
                "qT": np.ascontiguousarray(qT[sl]),
                "kT": np.ascontiguousarray(kT[sl]),
                "va": np.ascontiguousarray(va[sl]),
                "ptc": np.ascontiguousarray(ptc[sl]),
                "rpeR": rpeR,
                "idn": idn,
            }
        )
    return in_maps


def run(q, k, v, rpe_matrix, trace=False):
    nc = get_program()
    in_maps = prepare_inputs(q, k, v, rpe_matrix)
    res = run_bass_kernel_spmd(nc, in_maps, list(range(N_CORES)), trace=trace)
    outs = [res.results[c]["o"] for c in range(N_CORES)]
    o = np.concatenate(outs, axis=0).reshape(B, H, N, D)
    return o, res


def kernel(q, k, v, drop_noise=None, rpe_matrix=None, p=2, **kw):
    o, _ = run(q, k, v, rpe_matrix)
    return o


if __name__ == "__main__":
    rng = np.random.default_rng(0)
    q = rng.standard_normal((B, H, N, D), dtype=np.float32)
    k = rng.standard_normal((B, H, N, D), dtype=np.float32)
    v = rng.standard_normal((B, H, N, D), dtype=np.float32)
    rpe = rng.standard_normal((2 * N - 1, D), dtype=np.float32)
    o, _ = run(q, k, v, rpe)
    print("out", o.shape, o.dtype, np.abs(o).max())
